# revision 27
# baseline (speedup 1.0000x reference)
"""ConvAConnect TRN2 kernel: per-sample noisy-weight 3x3 conv, data-parallel over 8 cores.

Z[b] = conv2d_valid(X[b], W * Werr[loc_id[b]]) + bias * Berr[loc_id[b]]

Shapes: X[32,64,64,64] f32, W[3,3,64,128], bias[128], Werr[1000,3,3,64,128],
Berr[1000,128], loc_id[32] i32 -> Z[32,62,62,128] f32.

Strategy: shard batch (4 samples/core). Per the sharding hint, the per-sample
noisy weights memW = W*Werr[loc_id] and membias = bias*Berr[loc_id] are formed
host-side and sharded with the batch; X ships as fp16 cin-major X^T.

Device kernel per sample (fp16 operands, f32 PSUM accumulate):
  - ONE stacked SBUF tile XTs = [X^T; X^T shifted 64 pixels (one grid row)].
    The 9 conv taps become 6 K=128 matmuls per 512-pixel output chunk:
    3 row-pair blocks (fh 0+1, fw j) read XTs at offset j, and 3 single
    blocks (fh 2, fw j) read at offset 128+j with their lower 64 weight rows
    zeroed. Constant K=128 keeps the PE from switching tile config, and the
    single stack halves X DMA traffic vs a two-stack schedule (the DMA
    queues, ~100 GB/s each x3, are the scarce resource).
  - Output grid is 62 rows x 64 cols (2 junk columns keep width-64 alignment
    so every tap is a constant offset); junk columns are dropped on host.
  - PSUM drains fuse the per-sample bias add and emit fp16 into a
    [cout, grid] zbuf, alternating ScalarE (activation) / VectorE
    (tensor_scalar_add) per chunk; host does the final transpose.
  - The first two samples' loads are column-split across queues to cut
    first-matmul latency; the last output ships in halves to cut the tail.
"""

import sys
import numpy as np

for _p in ("/opt/trn_rl_repo", "/root/.axon_site"):
    if _p not in sys.path:
        sys.path.insert(0, _p)

N_CORES = 8
B = 32
PER_CORE = B // N_CORES
H = Wd = 64
CIN = 64
COUT = 128
HO = WO = 62
GRID = HO * 64          # 62 rows x 64 cols (2 junk cols/row)
XTL = 4104              # X^T free length: 4096 valid + pad (max read 4098)
XSRC = XTL + 64         # dram row length; +64 so the shifted lower half
                        # covers all XTL cols with host zeros (no stale SBUF)
NCHUNK = 512            # output-grid pixels per PSUM chunk (8 grid rows)
NCHUNKS = 8             # 7 full chunks + 1 of 384
NMM = 5                 # matmuls per chunk
WCAT = NMM * COUT       # wp(3*128) | wq(128) | ws(128, lower rows zero)

_compiled = {}


def _build():
    import concourse.mybir as mybir
    import concourse.tile as tile
    from concourse import bacc

    f32 = mybir.dt.float32
    f16 = mybir.dt.float16

    nc = bacc.Bacc("TRN2", target_bir_lowering=False, debug=False)

    xt_in = nc.dram_tensor("xt", [PER_CORE, CIN, XSRC], f16, kind="ExternalInput")
    mw_in = nc.dram_tensor("mw", [PER_CORE, 128, WCAT], f16, kind="ExternalInput")
    mb_in = nc.dram_tensor("mb", [COUT, PER_CORE], f32, kind="ExternalInput")
    z_out = nc.dram_tensor("z", [PER_CORE, 128, GRID], f16, kind="ExternalOutput")

    with tile.TileContext(nc) as tc:
        with (
            tc.tile_pool(name="const", bufs=1) as const,
            tc.tile_pool(name="xpool", bufs=3) as xpool,
            tc.tile_pool(name="wpool", bufs=3) as wpool,
            tc.tile_pool(name="zpool", bufs=3) as zpool,
            tc.tile_pool(name="psmm", bufs=4, space="PSUM") as psmm,
            tc.tile_pool(name="psw", bufs=1, space="PSUM") as psw,
        ):
            mb_all = const.tile([COUT, PER_CORE], f32, tag="mb")
            nc.sync.dma_start(mb_all[:], mb_in[:])

            # PE warm-up: a few throwaway matmuls on a zeroed scratch tile
            # ramp the Tensor engine to full p-state while the first
            # sample's inputs are still in flight
            warm = const.tile([128, NCHUNK], f16, tag="warm")
            nc.vector.memset(warm[:], 0.0)
            pw = psw.tile([128, NCHUNK], f32, tag="pw")
            for _ in range(7):
                nc.tensor.matmul(
                    pw[:], warm[:, 0:COUT], warm[:], start=True, stop=True
                )

            XQ = 1026  # column segment for the early samples' loads

            def load_sample(b, split):
                """DMA the two X stacks (xts1 = [X; X<<1], xts64 =
                [X; X<<64], both direct from HBM) and the noisy weights."""
                mw = wpool.tile([128, WCAT], f16, tag="mw")
                xts1 = xpool.tile([128, XTL], f16, tag="xts1")
                xts64 = xpool.tile([128, XTL], f16, tag="xts64")
                if split:
                    # early samples: column split across both HW queues so
                    # the first chunks' columns land fast; the weights ride
                    # right behind the first column segment
                    for q in range(4):
                        lo, hi = q * XQ, min((q + 1) * XQ, XTL)
                        nc.sync.dma_start(xts1[0:64, lo:hi], xt_in[b][:, lo:hi])
                        nc.scalar.dma_start(
                            xts1[64:128, lo:hi], xt_in[b][:, lo + 1 : hi + 1]
                        )
                        if q == 0:
                            eng = (nc.scalar, nc.sync)[b % 2]
                            eng.dma_start(mw[:], mw_in[b])
                        nc.sync.dma_start(
                            xts64[64:128, lo:hi], xt_in[b][:, lo + 64 : hi + 64]
                        )
                        nc.scalar.dma_start(xts64[0:64, lo:hi], xt_in[b][:, lo:hi])
                else:
                    nc.gpsimd.dma_start(mw[:], mw_in[b])
                    nc.sync.dma_start(xts1[0:64, :], xt_in[b][:, 0:XTL])
                    nc.sync.dma_start(xts1[64:128, :], xt_in[b][:, 1 : XTL + 1])
                    nc.scalar.dma_start(xts64[0:64, :], xt_in[b][:, 0:XTL])
                    nc.scalar.dma_start(xts64[64:128, :], xt_in[b][:, 64:XSRC])
                return xts1, xts64, mw

            samples = [load_sample(0, True), load_sample(1, True)]
            for b in range(PER_CORE):
                xts1, xts64, mw = samples[b]
                if b + 2 < PER_CORE:
                    samples.append(load_sample(b + 2, False))

                zbuf = zpool.tile([128, GRID], f16, tag="zbuf")

                for c in range(NCHUNKS):
                    base = c * NCHUNK
                    ncols = min(NCHUNK, GRID - base)
                    pc = psmm.tile([128, NCHUNK], f32, tag="pc")
                    # taps (fh,0)+(fh,1): K=128 pairs from the shift-1 stack
                    for fh in range(3):
                        nc.tensor.matmul(
                            pc[:, :ncols],
                            mw[:, fh * COUT : (fh + 1) * COUT],
                            xts1[:, base + fh * 64 : base + fh * 64 + ncols],
                            start=(fh == 0),
                            stop=False,
                        )
                    # taps (0,2)+(1,2): K=128 pair from the shift-64 stack
                    nc.tensor.matmul(
                        pc[:, :ncols],
                        mw[:, 3 * COUT : 4 * COUT],
                        xts64[:, base + 2 : base + 2 + ncols],
                        start=False,
                        stop=False,
                    )
                    # tap (2,2): K=128 with zero lower weight rows
                    nc.tensor.matmul(
                        pc[:, :ncols],
                        mw[:, 4 * COUT : 5 * COUT],
                        xts1[:, base + 130 : base + 130 + ncols],
                        start=False,
                        stop=True,
                    )
                    # drain PSUM -> zbuf fused with the per-sample bias add;
                    # all drains on VectorE keeps ScalarE a pure DMA engine
                    # (and drops its activation-table preamble load)
                    nc.vector.tensor_scalar_add(
                        zbuf[:, base : base + ncols],
                        pc[:, :ncols],
                        mb_all[:, b : b + 1],
                    )
                    if b == PER_CORE - 1 and c % 2 == 1:
                        # last sample ships per-quarter as chunks drain so
                        # the final DMA tail is a single 0.25 MB transfer
                        ZQ = GRID // 4
                        q = c // 2
                        eng = (nc.sync, nc.scalar)[q % 2]
                        eng.dma_start(
                            z_out[b][:, q * ZQ : (q + 1) * ZQ],
                            zbuf[:, q * ZQ : (q + 1) * ZQ],
                        )

                # ship the sample (host does the final transpose); the
                # last sample already shipped per-quarter inline above
                if b < PER_CORE - 1:
                    eng = (nc.gpsimd, nc.sync, nc.scalar)[b]
                    eng.dma_start(z_out[b], zbuf[:])

    nc.compile()
    return nc


def _get_nc():
    if "nc" not in _compiled:
        _compiled["nc"] = _build()
    return _compiled["nc"]


def _prep_inputs(X, W, bias, Werr, Berr, loc_id):
    """Host-side shard/layout prep. Returns per-core in_maps."""
    X = np.asarray(X, dtype=np.float32)
    W = np.asarray(W, dtype=np.float32)
    bias = np.asarray(bias, dtype=np.float32)
    Werr = np.asarray(Werr, dtype=np.float32)
    Berr = np.asarray(Berr, dtype=np.float32)
    loc_id = np.asarray(loc_id)

    # X^T: [B, CIN, H*W] zero-padded to XSRC, fp16
    xt = np.zeros((B, CIN, XSRC), dtype=np.float16)
    xt[:, :, : H * Wd] = X.transpose(0, 3, 1, 2).reshape(B, CIN, H * Wd)

    # memW = W * Werr[loc_id], laid out as [128, 640]:
    #   wp block fh: rows = [memW[fh, 0, cin, :]; memW[fh, 1, cin, :]]
    #   wq block: rows = [memW[0, 2, cin, :]; memW[1, 2, cin, :]]
    #   ws block: rows = [memW[2, 2, cin, :]; zeros]
    def cat_blocks(w):
        lead = w.shape[:-4]
        out = np.zeros(lead + (128, WCAT), dtype=np.float16)
        # [..., fh, fw2, cin, cout] -> [..., fw2, cin, fh, cout] -> [128, 384]
        out[..., :, 0 : 3 * COUT] = np.moveaxis(w[..., :, :2, :, :], -4, -2).reshape(
            lead + (128, 3 * COUT)
        )
        out[..., :, 3 * COUT : 4 * COUT] = w[..., :2, 2, :, :].reshape(
            lead + (128, COUT)
        )
        out[..., 0:64, 4 * COUT : 5 * COUT] = w[..., 2, 2, :, :]
        return out

    mwcat = cat_blocks(W[None] * Werr[loc_id])   # [B, 128, 768] fp16
    mb = (bias[None] * Berr[loc_id]).astype(np.float32)  # [B, 128]

    in_maps = []
    for i in range(N_CORES):
        s = slice(i * PER_CORE, (i + 1) * PER_CORE)
        in_maps.append(
            {
                "xt": np.ascontiguousarray(xt[s]),
                "mw": np.ascontiguousarray(mwcat[s]),
                "mb": np.ascontiguousarray(mb[s].T),
            }
        )
    return in_maps


def _run(in_maps, trace=False, **kw):
    from concourse.bass_utils import run_bass_kernel_spmd

    nc = _get_nc()
    return run_bass_kernel_spmd(nc, in_maps, list(range(N_CORES)), trace=trace, **kw)


def _unshard(results):
    zb = np.concatenate([results[i]["z"] for i in range(N_CORES)], axis=0)
    # zb[b, cout, ho*64+wo] -> Z[b, ho, wo, cout]
    v = zb.astype(np.float32).reshape(B, COUT, HO, 64).transpose(0, 2, 3, 1)
    return np.ascontiguousarray(v[:, :, :WO, :])


def kernel(X, W, bias, Werr, Berr, loc_id):
    in_maps = _prep_inputs(X, W, bias, Werr, Berr, loc_id)
    res = _run(in_maps)
    return _unshard(res.results)


# revision 28
# speedup vs baseline: 1.1389x; 1.1389x over previous
"""ConvAConnect TRN2 kernel: per-sample noisy-weight 3x3 conv, data-parallel over 8 cores.

Z[b] = conv2d_valid(X[b], W * Werr[loc_id[b]]) + bias * Berr[loc_id[b]]

Shapes: X[32,64,64,64] f32, W[3,3,64,128], bias[128], Werr[1000,3,3,64,128],
Berr[1000,128], loc_id[32] i32 -> Z[32,62,62,128] f32.

Strategy: shard batch (4 samples/core). Per the sharding hint, the per-sample
noisy weights memW = W*Werr[loc_id] and membias = bias*Berr[loc_id] are formed
host-side and sharded with the batch; X ships as fp16 cin-major X^T.

Device kernel per sample (fp16 operands, f32 PSUM accumulate):
  - ONE stacked SBUF tile XTs = [X^T; X^T shifted 64 pixels (one grid row)].
    The 9 conv taps become 6 K=128 matmuls per 512-pixel output chunk:
    3 row-pair blocks (fh 0+1, fw j) read XTs at offset j, and 3 single
    blocks (fh 2, fw j) read at offset 128+j with their lower 64 weight rows
    zeroed. Constant K=128 keeps the PE from switching tile config, and the
    single stack halves X DMA traffic vs a two-stack schedule (the DMA
    queues, ~100 GB/s each x3, are the scarce resource).
  - Output grid is 62 rows x 64 cols (2 junk columns keep width-64 alignment
    so every tap is a constant offset); junk columns are dropped on host.
  - PSUM drains fuse the per-sample bias add and emit fp16 into a
    [cout, grid] zbuf, alternating ScalarE (activation) / VectorE
    (tensor_scalar_add) per chunk; host does the final transpose.
  - The first two samples' loads are column-split across queues to cut
    first-matmul latency; the last output ships in halves to cut the tail.
"""

import sys
import numpy as np

for _p in ("/opt/trn_rl_repo", "/root/.axon_site"):
    if _p not in sys.path:
        sys.path.insert(0, _p)

N_CORES = 8
B = 32
PER_CORE = B // N_CORES
H = Wd = 64
CIN = 64
COUT = 128
HO = WO = 62
GRID = HO * 64          # 62 rows x 64 cols (2 junk cols/row)
XTL = 4104              # X^T free length: 4096 valid + pad (max read 4098)
XSRC = XTL + 64         # dram row length; +64 so the shifted lower half
                        # covers all XTL cols with host zeros (no stale SBUF)
NCHUNK = 512            # output-grid pixels per PSUM chunk (8 grid rows)
NCHUNKS = 8             # 7 full chunks + 1 of 384
NMM = 6                 # matmuls per chunk
WCAT = NMM * COUT       # 3 pair blocks | 3 single blocks (lower rows zero)

_compiled = {}


def _build():
    import concourse.mybir as mybir
    import concourse.tile as tile
    from concourse import bacc

    f32 = mybir.dt.float32
    f16 = mybir.dt.float16

    nc = bacc.Bacc("TRN2", target_bir_lowering=False, debug=False)

    xt_in = nc.dram_tensor("xt", [PER_CORE, CIN, XSRC], f16, kind="ExternalInput")
    mw_in = nc.dram_tensor("mw", [PER_CORE, 128, WCAT], f16, kind="ExternalInput")
    mb_in = nc.dram_tensor("mb", [COUT, PER_CORE], f32, kind="ExternalInput")
    z_out = nc.dram_tensor("z", [PER_CORE, 128, GRID], f16, kind="ExternalOutput")

    with tile.TileContext(nc) as tc:
        with (
            tc.tile_pool(name="const", bufs=1) as const,
            tc.tile_pool(name="xpool", bufs=3) as xpool,
            tc.tile_pool(name="wpool", bufs=3) as wpool,
            tc.tile_pool(name="zpool", bufs=3) as zpool,
            tc.tile_pool(name="psmm", bufs=4, space="PSUM") as psmm,
            tc.tile_pool(name="psw", bufs=1, space="PSUM") as psw,
        ):
            mb_all = const.tile([COUT, PER_CORE], f32, tag="mb")
            nc.sync.dma_start(mb_all[:], mb_in[:])

            # PE warm-up: a few throwaway matmuls on a zeroed scratch tile
            # ramp the Tensor engine to full p-state while the first
            # sample's inputs are still in flight
            warm = const.tile([128, NCHUNK], f16, tag="warm")
            nc.vector.memset(warm[:], 0.0)
            pw = psw.tile([128, NCHUNK], f32, tag="pw")
            for _ in range(7):
                nc.tensor.matmul(
                    pw[:], warm[:, 0:COUT], warm[:], start=True, stop=True
                )

            XQ = 1026  # column segment for the early samples' loads

            def load_sample(b, split):
                """DMA the stacked X tile (upper = X^T, lower = X^T<<64)
                and this sample's noisy weights."""
                mw = wpool.tile([128, WCAT], f16, tag="mw")
                xts = xpool.tile([128, XTL], f16, tag="xts")
                if split:
                    # early samples: fine column split across both HW
                    # queues so the first chunks' columns land fast; the
                    # weights ride right behind the first column segment,
                    # half on each queue
                    for q in range(4):
                        lo, hi = q * XQ, min((q + 1) * XQ, XTL)
                        nc.sync.dma_start(xts[0:64, lo:hi], xt_in[b][:, lo:hi])
                        nc.scalar.dma_start(
                            xts[64:128, lo:hi], xt_in[b][:, lo + 64 : hi + 64]
                        )
                        if q == 0:
                            eng = (nc.scalar, nc.sync)[b % 2]
                            eng.dma_start(mw[:], mw_in[b])
                else:
                    nc.gpsimd.dma_start(mw[:], mw_in[b])
                    nc.sync.dma_start(xts[0:64, :], xt_in[b][:, 0:XTL])
                    nc.scalar.dma_start(xts[64:128, :], xt_in[b][:, 64:XSRC])
                return xts, mw

            samples = [load_sample(0, True), load_sample(1, True)]
            for b in range(PER_CORE):
                xts, mw = samples[b]
                if b + 2 < PER_CORE:
                    samples.append(load_sample(b + 2, False))

                zbuf = zpool.tile([128, GRID], f16, tag="zbuf")

                for c in range(NCHUNKS):
                    base = c * NCHUNK
                    ncols = min(NCHUNK, GRID - base)
                    pc = psmm.tile([128, NCHUNK], f32, tag="pc")
                    # taps (0,j)+(1,j): K=128 row pairs from the stack
                    for j in range(3):
                        nc.tensor.matmul(
                            pc[:, :ncols],
                            mw[:, j * COUT : (j + 1) * COUT],
                            xts[:, base + j : base + j + ncols],
                            start=(j == 0),
                            stop=False,
                        )
                    # taps (2,j): K=128 with zero lower weight rows
                    for j in range(3):
                        nc.tensor.matmul(
                            pc[:, :ncols],
                            mw[:, (3 + j) * COUT : (4 + j) * COUT],
                            xts[:, base + 128 + j : base + 128 + j + ncols],
                            start=False,
                            stop=(j == 2),
                        )
                    # drain PSUM -> zbuf fused with the per-sample bias add;
                    # all drains on VectorE keeps ScalarE a pure DMA engine
                    # (and drops its activation-table preamble load)
                    nc.vector.tensor_scalar_add(
                        zbuf[:, base : base + ncols],
                        pc[:, :ncols],
                        mb_all[:, b : b + 1],
                    )
                    if b == PER_CORE - 1 and c % 2 == 1:
                        # last sample ships per-quarter as chunks drain so
                        # the final DMA tail is a single 0.25 MB transfer
                        ZQ = GRID // 4
                        q = c // 2
                        eng = (nc.sync, nc.scalar)[q % 2]
                        eng.dma_start(
                            z_out[b][:, q * ZQ : (q + 1) * ZQ],
                            zbuf[:, q * ZQ : (q + 1) * ZQ],
                        )

                # ship the sample (host does the final transpose); the
                # last sample already shipped per-quarter inline above
                if b < PER_CORE - 1:
                    eng = (nc.gpsimd, nc.sync, nc.scalar)[b]
                    eng.dma_start(z_out[b], zbuf[:])

    nc.compile()
    return nc


def _get_nc():
    if "nc" not in _compiled:
        _compiled["nc"] = _build()
    return _compiled["nc"]


def _prep_inputs(X, W, bias, Werr, Berr, loc_id):
    """Host-side shard/layout prep. Returns per-core in_maps."""
    X = np.asarray(X, dtype=np.float32)
    W = np.asarray(W, dtype=np.float32)
    bias = np.asarray(bias, dtype=np.float32)
    Werr = np.asarray(Werr, dtype=np.float32)
    Berr = np.asarray(Berr, dtype=np.float32)
    loc_id = np.asarray(loc_id)

    # X^T: [B, CIN, H*W] zero-padded to XSRC, fp16
    xt = np.zeros((B, CIN, XSRC), dtype=np.float16)
    xt[:, :, : H * Wd] = X.transpose(0, 3, 1, 2).reshape(B, CIN, H * Wd)

    # memW = W * Werr[loc_id], laid out as [128, 768]:
    #   pair block j: rows = [memW[0, j, cin, :]; memW[1, j, cin, :]]
    #   single block j: rows = [memW[2, j, cin, :]; zeros]
    def cat_blocks(w):
        lead = w.shape[:-4]
        out = np.zeros(lead + (128, WCAT), dtype=np.float16)
        # [..., fh2, fw, cin, cout] -> [..., fw, fh2*cin, cout]
        pair = np.moveaxis(w[..., 0:2, :, :, :], -3, -4).reshape(
            lead + (3, 128, COUT)
        )
        for j in range(3):
            out[..., :, j * COUT : (j + 1) * COUT] = pair[..., j, :, :]
            out[..., 0:64, (3 + j) * COUT : (4 + j) * COUT] = w[..., 2, j, :, :]
        return out

    mwcat = cat_blocks(W[None] * Werr[loc_id])   # [B, 128, 768] fp16
    mb = (bias[None] * Berr[loc_id]).astype(np.float32)  # [B, 128]

    in_maps = []
    for i in range(N_CORES):
        s = slice(i * PER_CORE, (i + 1) * PER_CORE)
        in_maps.append(
            {
                "xt": np.ascontiguousarray(xt[s]),
                "mw": np.ascontiguousarray(mwcat[s]),
                "mb": np.ascontiguousarray(mb[s].T),
            }
        )
    return in_maps


def _run(in_maps, trace=False, **kw):
    from concourse.bass_utils import run_bass_kernel_spmd

    nc = _get_nc()
    return run_bass_kernel_spmd(nc, in_maps, list(range(N_CORES)), trace=trace, **kw)


def _unshard(results):
    zb = np.concatenate([results[i]["z"] for i in range(N_CORES)], axis=0)
    # zb[b, cout, ho*64+wo] -> Z[b, ho, wo, cout]
    v = zb.astype(np.float32).reshape(B, COUT, HO, 64).transpose(0, 2, 3, 1)
    return np.ascontiguousarray(v[:, :, :WO, :])


def kernel(X, W, bias, Werr, Berr, loc_id):
    in_maps = _prep_inputs(X, W, bias, Werr, Berr, loc_id)
    res = _run(in_maps)
    return _unshard(res.results)
